# revision 24
# baseline (speedup 1.0000x reference)
"""Causal single-head attention (B=4, T=4096, C=1024, H=64) on 8 trn2 NeuronCores.

Sharding: core = (batch b = core//2, parity p = core%2). Each core owns the
interleaved context tiles {p, p+2, ...} of its batch (balanced under the causal
mask) and computes partial flash-attention (numerator + denominator) for ALL
queries of the batch; the host sums the two partials per batch and divides.

Host-side prep is layout only: the x shard is passed pre-transposed and
slice-major (each 512-column slice is one contiguous [128, 4096] block so its
DMA is 128 8KB runs), weights pre-fused into their SBUF layout, causal masks
as data. All numerics run on device. No collectives.

Device pipeline per core:
  warmup matmul stream (HAM un-throttle) while x^T slices stream in on two
  HWDGE queues -> project [S^|G^] with a fused [wk|wq] stationary, evicting
  to fp8 (fp8 matmuls run at bf16 speed; operands/dups are half size) ->
  per 512-query block: row-packed fp8 score matmuls -> exp on ACT (scale
  folded in) -> data-driven causal masks on DVE -> bf16 PV matmuls (V padded
  to 128 cols) accumulating [V|1]^T @ P^T in a single PSUM chain -> write
  O^T_aug [H+1, T].

Query columns are processed in [own|partner] order per 512-block; the host
maps them back to absolute order per core in combine_outputs.
"""

import sys

for _p in ("/root/.axon_site/_ro/trn_rl_repo", "/root/.axon_site/_ro/pypackages"):
    if _p not in sys.path:
        sys.path.append(_p)

import ml_dtypes
import numpy as np

import concourse.bass as bass
import concourse.mybir as mybir
import concourse.tile as tile
from concourse import bacc
from concourse.bass_utils import run_bass_kernel_spmd
from concourse.masks import make_identity

B, T, C, H = 4, 4096, 1024, 64
N_CORES = 8
SCALE = C ** -0.5
F32 = mybir.dt.float32
BF16 = mybir.dt.bfloat16
FP8 = mybir.dt.float8e4


def build_kernel(t_full=T):
    """Build the SPMD Bass/Tile program for sequence length t_full."""
    t_own = t_full // 2           # context rows owned by this core
    n_own = t_own // 128          # own 128-row s-tiles
    n_cchunk = C // 128           # contraction chunks of 128
    n_tq = t_full // 512          # projection/load slices (stored order)
    n_vq = t_own // 512           # V projection slices (own region)

    nc = bacc.Bacc("TRN2", target_bir_lowering=False, debug=False,
                   num_devices=N_CORES)

    xt_d = nc.dram_tensor("xt_own", [128, n_cchunk * t_full // 2], BF16,
                      kind="ExternalInput").ap()
    xt8_d = nc.dram_tensor("xt_part", [128, n_cchunk * t_full // 2], FP8,
                           kind="ExternalInput").ap()
    wkq8_d = nc.dram_tensor("wkq8", [128, n_cchunk * 128], FP8,
                            kind="ExternalInput").ap()
    aux_w = n_cchunk * 128 + n_cchunk * H + 1024
    aux_d = nc.dram_tensor("aux", [128, aux_w], BF16,
                           kind="ExternalInput").ap()
    bkq_d = nc.dram_tensor("bkq", [128], F32, kind="ExternalInput").ap()
    bv_d = nc.dram_tensor("bv", [128], F32, kind="ExternalInput").ap()
    out_d = nc.dram_tensor("out_part", [H + 1, t_full], F32,
                           kind="ExternalOutput").ap()

    with tile.TileContext(nc) as tc:
        with (
            tc.tile_pool(name="persist", bufs=1) as pp,
            tc.tile_pool(name="psB", bufs=2, space="PSUM") as psb,
            tc.tile_pool(name="psS", bufs=2, space="PSUM") as pss,
            tc.tile_pool(name="psO", bufs=2, space="PSUM") as pso,
            tc.tile_pool(name="ptp", bufs=4) as ptp,
            tc.tile_pool(name="outp", bufs=2) as outp,
        ):
            # ---- persistent SBUF tensors ----
            # x^T, slice-major: slice tq at cols 4096*tq, chunk j at
            # +512*j. Own region bf16 (feeds K/Q/V); partner region fp8
            # (feeds K/Q only -- error identical to the fp8 eviction)
            xt = pp.tile([128, n_cchunk * t_full // 2], BF16)
            xt8 = pp.tile([128, n_cchunk * t_full // 2], FP8)
            wkq8_sb = pp.tile([128, n_cchunk * 128], FP8)
            kqT = pp.tile([128, t_full], FP8)             # 0:64 = S^T, 64:128 = G^T
            sd_hi = pp.tile([128, t_full], FP8)           # S^T dup at partitions 64:128
            gt_lo = pp.tile([64, t_own], FP8)             # G^T (own) at partitions 0:64
            vT = pp.tile([128, t_own // 2], BF16)         # V^T pair-packed
            v_sb = pp.tile([128, n_own * 128], BF16)      # V_aug tiles [128,128]
            aux_sb = pp.tile([128, aux_w], BF16)
            wkq_sb = aux_sb[:, 0:n_cchunk * 128]
            wv_sb = aux_sb[:, n_cchunk * 128:n_cchunk * 128 + n_cchunk * H]
            mask0 = aux_sb[:, aux_w - 1024:aux_w - 512]
            mask1 = aux_sb[:, aux_w - 512:aux_w]
            bias_kq = pp.tile([128, 1], F32)
            bias_v = pp.tile([128, 1], F32)
            ident = pp.tile([128, 128], BF16)
            swap8 = pp.tile([128, 128], FP8)              # half-swapped identity
            wt = pp.tile([128, 512], BF16)                # warmup operand

            make_identity(nc, ident[:, :])
            # swap8^T @ m rotates partitions by 64: out[i] = m[(i+64)%128]
            nc.vector.tensor_copy(swap8[:, 0:64], ident[:, 64:128])
            nc.vector.tensor_copy(swap8[:, 64:128], ident[:, 0:64])

            xt_sl = xt[:, :].rearrange("p (q j n) -> p q j n",
                                       j=n_cchunk, n=512)
            xd_sl = xt_d.rearrange("p (q j n) -> p q j n", j=n_cchunk, n=512)
            xt8_sl = xt8[:, :].rearrange("p (q j n) -> p q j n",
                                         j=n_cchunk, n=512)
            xd8_sl = xt8_d.rearrange("p (q j n) -> p q j n",
                                     j=n_cchunk, n=512)

            # warmup stream: keep the PE busy while x streams in so HAM
            # un-throttles before the first real matmul (zeros x zeros)
            nc.gpsimd.memset(wt[:, :], 0.0)
            for w in range(8):
                pool = psb if w % 2 else pso
                pw = pool.tile([128, 512], F32, tag="psB" if w % 2 else "psO")
                nc.tensor.matmul(pw[:, :], wt[:, 0:128], wt[:, :],
                                 start=True, stop=True)

            # weights + small operands first on the scalar HWDGE queue
            nc.scalar.dma_start(out=aux_sb[:, :], in_=aux_d)
            nc.scalar.dma_start(out=wkq8_sb[:, :], in_=wkq8_d)
            nc.scalar.dma_start(out=bias_kq[:, 0:1], in_=bkq_d[:, None])
            nc.scalar.dma_start(out=bias_v[:, 0:1], in_=bv_d[:, None])

            # x slices (contiguous blocks), ordered by first use; partner
            # slices are fp8 (half bytes). Slice 0 split so projections
            # start on its first half.
            nc.sync.dma_start(out=xt_sl[:, 0, 0:4, :],
                              in_=xd_sl[:, 0, 0:4, :])
            nc.sync.dma_start(out=xt_sl[:, 0, 4:8, :],
                              in_=xd_sl[:, 0, 4:8, :])
            nc.sync.dma_start(out=xt8_sl[:, 0], in_=xd8_sl[:, 0])
            for tq in (1,):
                nc.sync.dma_start(out=xt_sl[:, tq], in_=xd_sl[:, tq])
            nc.sync.dma_start(out=xt8_sl[:, 1], in_=xd8_sl[:, 1])
            for tq in (2,):
                nc.sync.dma_start(out=xt_sl[:, tq], in_=xd_sl[:, tq])
            nc.scalar.dma_start(out=xt8_sl[:, 2], in_=xd8_sl[:, 2])
            for tq in (3,):
                nc.scalar.dma_start(out=xt_sl[:, tq], in_=xd_sl[:, tq])
            nc.scalar.dma_start(out=xt8_sl[:, 3], in_=xd8_sl[:, 3])

            # V_aug padding: zero cols 64:128 of each slot, ones at col 64
            nc.gpsimd.memset(v_sb[:, :], 0.0)
            nc.vector.tensor_scalar(
                v_sb[:, :].rearrange("p (i c) -> p i c", c=128)[:, :, 64],
                ident[:, 0:n_own], 0.0, 1.0,
                op0=mybir.AluOpType.mult, op1=mybir.AluOpType.add)

            def filler(n):
                # zero matmuls that keep the PE array active through DMA
                # waits so the HAM clock gate stays at 8/8
                for _ in range(n):
                    fp = psb.tile([128, 128], F32, tag="psB")
                    nc.tensor.matmul(fp[:, :], wt[:, 0:128], wt[:, 0:128],
                                     start=True, stop=True)

            def kq_closures(tq):
                # the 8 chunk matmuls + the evict/duplicate step as
                # closures so they can interleave into attention stages
                st = {}

                own = tq < n_vq
                w_sb = wkq_sb if own else wkq8_sb
                x_v = xt_sl if own else xt8_sl
                lq = tq if own else tq - n_vq

                def mk(j):
                    def mm():
                        if j == 0:
                            ps_kq = psb.tile([128, 512], F32, tag="psB")
                            st["ps"] = ps_kq
                        nc.tensor.matmul(
                            st["ps"][:, :], w_sb[:, 128 * j:128 * (j + 1)],
                            x_v[:, lq, j, :],
                            start=(j == 0), stop=(j == n_cchunk - 1))
                    return mm

                def finish():
                    nc.vector.tensor_scalar_add(
                        kqT[:, 512 * tq:512 * (tq + 1)], st["ps"][:, :],
                        bias_kq[:, 0:1])
                    # partition relocation on the PE (one matmul against the
                    # half-swapped identity) instead of gpsimd SWDGE copies,
                    # whose ~100ns/descriptor costs ~6.4us of latency each
                    pr = psb.tile([128, 512], F32, tag="psB")
                    nc.tensor.matmul(
                        pr[:, :], swap8[:, :],
                        kqT[:, 512 * tq:512 * (tq + 1)],
                        start=True, stop=True)
                    if tq < n_vq:
                        # G^T copy into partitions 0:64 (own region only)
                        nc.vector.tensor_copy(
                            gt_lo[:, 512 * tq:512 * (tq + 1)], pr[0:64, :])
                    # S^T copy into partitions 64:128 for row-packed scores
                    nc.vector.tensor_copy(
                        sd_hi[64:128, 512 * tq:512 * (tq + 1)], pr[64:128, :])

                return [mk(j) for j in range(n_cchunk)] + [finish]

            def project_kq(tq):
                for c in kq_closures(tq):
                    c()

            def project_v_pair(m):
                # col-tiled: slice 2m into out partitions 0:64 (array cols
                # 0:64), slice 2m+1 into 64:128 (cols 64:128) — concurrent
                ps = psb.tile([128, 512], F32, tag="psB")
                for j in range(n_cchunk):
                    nc.tensor.matmul(
                        ps[0:64, :], wv_sb[:, H * j:H * (j + 1)],
                        xt_sl[:, 2 * m, j, :],
                        start=(j == 0), stop=(j == n_cchunk - 1),
                        tile_position=(0, 0), skip_group_check=True)
                    nc.tensor.matmul(
                        ps[64:128, :], wv_sb[:, H * j:H * (j + 1)],
                        xt_sl[:, 2 * m + 1, j, :],
                        start=(j == 0), stop=(j == n_cchunk - 1),
                        tile_position=(0, 64), skip_group_check=True)
                nc.vector.tensor_scalar_add(
                    vT[:, 512 * m:512 * (m + 1)], ps[:, :], bias_v[:, 0:1])

            vsb4 = v_sb[:, :].rearrange("p (m g c e) -> p m g c e",
                                        g=2, c=4, e=128)

            def v_transpose_pair(m, c):
                # vT[:, pair m, col tile c] = [V^T tile 8m+c ; V^T tile
                # 8m+4+c] stacked on partitions -> one [128,128] transpose
                ps = psb.tile([128, 128], BF16, tag="psB")
                nc.tensor.transpose(
                    ps[:, :], vT[:, 512 * m + 128 * c:512 * m + 128 * (c + 1)],
                    ident[:, :])
                nc.vector.tensor_copy(
                    vsb4[:, m, :, c, 0:64],
                    ps[:, :].rearrange("p (g e) -> p g e", e=64))

            kq_lo = kqT[0:64, :].rearrange("p (h t) -> p h t", h=2)
            sd_v = sd_hi[64:128, :].rearrange("p (h t) -> p h t", h=2)

            def emit_scores(tb, ip, packed=True):
                ps = pss.tile([128, 1024], F32, tag="psS")
                i0, i1 = 2 * ip, 2 * ip + 1
                nc.tensor.matmul(
                    ps[:, 0:512],
                    gt_lo[:, 128 * i0:128 * (i0 + 1)],
                    kq_lo[:, :, 256 * tb:256 * (tb + 1)],
                    start=True, stop=True, tile_position=(0, 0))
                if packed:
                    # concurrent with MM1 in the upper array half; needs
                    # the sd_hi duplicate of this block's slices
                    nc.tensor.matmul(
                        ps[:, 512:1024],
                        kqT[64:128, 128 * i1:128 * (i1 + 1)],
                        sd_v[:, :, 256 * tb:256 * (tb + 1)],
                        start=True, stop=True, tile_position=(64, 0))
                else:
                    # serial variant on the lower half: avoids the sd_hi
                    # duplicate DMA on stage-boundary critical paths
                    nc.tensor.matmul(
                        ps[:, 512:1024],
                        gt_lo[:, 128 * i1:128 * (i1 + 1)],
                        kq_lo[:, :, 256 * tb:256 * (tb + 1)],
                        start=True, stop=True, tile_position=(0, 0))
                return ps

            def attention_blocks(tbs, ps_pre=None, bgq=None, npop=2,
                                 unpacked_until=0):
                # software-pipelined: scores for later steps are emitted
                # BEFORE the PV matmuls of step idx, so the PE never sits
                # behind a PV that is waiting on the exp of its own step
                seq = [(tb, ip) for tb in tbs for ip in range(tb + 1)]
                po = {}
                psq = list(ps_pre) if ps_pre else [emit_scores(*seq[0])]
                for idx, (tb, ip) in enumerate(seq):
                    i0, i1 = 2 * ip, 2 * ip + 1
                    ps = psq.pop(0)
                    pt = ptp.tile([128, 1024], BF16, tag="pt")
                    nc.scalar.activation(
                        pt[:, :], ps[:, :],
                        mybir.ActivationFunctionType.Exp, scale=SCALE)
                    nxt = idx + len(psq) + 1
                    if nxt < len(seq):
                        psq.append(emit_scores(*seq[nxt],
                                               packed=(nxt >= unpacked_until)))
                    if ip == tb:
                        nc.vector.tensor_mul(
                            pt[:, 0:512], pt[:, 0:512], mask0)
                        nc.vector.tensor_mul(
                            pt[:, 512:1024], pt[:, 512:1024], mask1)
                    if ip == 0:
                        po_t = pso.tile([128, 512], F32, tag="psO")
                        po[tb] = po_t
                    nc.tensor.matmul(
                        po[tb][:, :], v_sb[:, 128 * i0:128 * (i0 + 1)],
                        pt[:, 0:512], start=(ip == 0), stop=False)
                    nc.tensor.matmul(
                        po[tb][:, :], v_sb[:, 128 * i1:128 * (i1 + 1)],
                        pt[:, 512:1024], start=False, stop=(ip == tb))
                    if bgq:
                        for _ in range(min(npop, len(bgq))):
                            bgq.pop(0)()
                    if ip == tb:
                        ob = outp.tile([65, 512], F32, tag="ob")
                        nc.vector.tensor_copy(ob[:, :], po[tb][0:65, :])
                        nc.sync.dma_start(
                            out=out_d[:, 512 * tb:512 * (tb + 1)],
                            in_=ob[:, :])

            # staged pipeline: stage k runs the two query blocks its
            # slices complete; the NEXT stage's K/Q projections interleave
            # into this stage's attention via the background queue, and
            # filler matmuls bridge early DMA waits (HAM stays at 8/8)
            bgq = []
            for k in range(n_vq):
                if k == 0:
                    filler(3)
                    cl0 = kq_closures(0)
                    for c in cl0[:4]:
                        c()
                    filler(12)   # bridges the slice-0 second-half DMA wait
                    for c in cl0[4:]:
                        c()
                    project_kq(n_vq)
                    filler(4)
                else:
                    while bgq:
                        bgq.pop(0)()
                    filler(2)
                if k == n_vq - 1:
                    tbs = [2 * k + 1, 2 * k]
                else:
                    tbs = [2 * k, 2 * k + 1]
                seq = [(tb, ip) for tb in tbs for ip in range(tb + 1)]
                # unpacked prologue scores: no sd_hi-duplicate dependency
                # on the stage-boundary critical path
                ps_pre = [emit_scores(*seq[0], packed=False),
                          emit_scores(*seq[1], packed=False)]
                if k % 2 == 0:
                    project_v_pair(k // 2)
                    for c in range(4):
                        v_transpose_pair(k // 2, c)
                if k + 1 < n_vq:
                    bgq += kq_closures(k + 1) + kq_closures(n_vq + k + 1)
                attention_blocks(tbs, ps_pre, bgq, npop=4 if k == 0 else 2,
                                 unpacked_until=3 if k == 0 else 0)

    nc.compile()
    return nc


def make_core_inputs(x, Wk, bk, Wq, bq, Wv, bv, t_full=T):
    """Shard FULL inputs into the 8 per-core input dicts (layout prep only)."""
    n_tiles = t_full // 128
    n_cchunk = C // 128
    Wk = np.asarray(Wk, np.float32)
    Wq = np.asarray(Wq, np.float32)
    Wv = np.asarray(Wv, np.float32)
    wkq = np.empty((128, n_cchunk * 128), np.float32)
    wvf = np.empty((128, n_cchunk * H), np.float32)
    for j in range(n_cchunk):
        wkq[:, 128 * j:128 * j + 64] = Wk[128 * j:128 * (j + 1), :]
        wkq[:, 128 * j + 64:128 * (j + 1)] = Wq[128 * j:128 * (j + 1), :]
        wvf[:, H * j:H * (j + 1)] = Wv[128 * j:128 * (j + 1), :]
    bkq = np.concatenate([np.asarray(bk, np.float32),
                          np.asarray(bq, np.float32)])
    ins = []
    for core in range(N_CORES):
        b, p = core // 2, core % 2
        own = np.concatenate(
            [x[b, 128 * j:128 * (j + 1), :] for j in range(p, n_tiles, 2)]
            + [x[b, 128 * j:128 * (j + 1), :]
               for j in range(1 - p, n_tiles, 2)],
            axis=0)
        # slice-major x^T: slice tq -> [128, 8*512] block, chunk j at 512*j
        ownT = np.ascontiguousarray(own.T)            # [C, t_full]
        xsl = ownT.reshape(n_cchunk, 128, t_full // 512, 512)
        xsl = xsl.transpose(1, 2, 0, 3).reshape(128, n_cchunk * t_full)
        half = n_cchunk * t_full // 2
        # mask[m][r, c]: own s-tile (local parity m, abs tile 4tb+2m+p) vs
        # query sub-tile c//128 (abs tile 4tb + A[c//128]); valid iff s <= t
        A = [p, 2 + p, 1 - p, 3 - p]
        masks = np.zeros((2, 128, 512), np.float32)
        rr = np.arange(128)[:, None]
        for m in (0, 1):
            for sub in range(4):
                cz = np.arange(128)[None, :]
                s_abs = 128 * (2 * m + p) + rr
                t_abs = 128 * A[sub] + cz
                masks[m, :, 128 * sub:128 * (sub + 1)] = (s_abs <= t_abs)
        aux = np.concatenate([wkq, wvf, masks[0], masks[1]], axis=1)
        ins.append({
            "xt_own": xsl[:, :half].astype(ml_dtypes.bfloat16),
            "xt_part": xsl[:, half:].astype(ml_dtypes.float8_e4m3fn),
            "wkq8": wkq.astype(ml_dtypes.float8_e4m3fn),
            "aux": aux.astype(ml_dtypes.bfloat16),
            "bkq": bkq,
            "bv": np.concatenate([np.asarray(bv, np.float32)] * 2),
        })
    return ins


def _col_perm(p, t_full):
    """stored column -> absolute t index for a core with parity p."""
    A = [p, 2 + p, 1 - p, 3 - p]
    perm = np.empty(t_full, np.int64)
    for tb in range(t_full // 512):
        for sub in range(4):
            a = 128 * (4 * tb + A[sub])
            s = 512 * tb + 128 * sub
            perm[s:s + 128] = np.arange(a, a + 128)
    return perm


def combine_outputs(parts, t_full=T):
    """parts: list of 8 arrays [H+1, t_full] -> full output [B, t_full, H]."""
    out = np.empty((B, t_full, H), np.float32)
    for b in range(B):
        acc = np.zeros((H + 1, t_full), np.float32)
        for p in (0, 1):
            perm = _col_perm(p, t_full)
            acc[:, perm] += parts[2 * b + p]
        out[b] = (acc[:H, :] / acc[H:H + 1, :]).T
    return out


_NC_CACHE = {}


def kernel(x, Wk, bk, Wq, bq, Wv, bv):
    x = np.asarray(x, np.float32)
    t_full = x.shape[1]
    if t_full not in _NC_CACHE:
        _NC_CACHE[t_full] = build_kernel(t_full)
    nc = _NC_CACHE[t_full]
    ins = make_core_inputs(x, Wk, bk, Wq, bq, Wv, bv, t_full)
    res = run_bass_kernel_spmd(nc, ins, list(range(N_CORES)))
    parts = [res.results[i]["out_part"] for i in range(N_CORES)]
    return combine_outputs(parts, t_full)


if __name__ == "__main__":
    rng = np.random.default_rng(0)
    x = rng.standard_normal((B, T, C), dtype=np.float32)
    Wk = rng.standard_normal((C, H), dtype=np.float32) * SCALE
    Wq = rng.standard_normal((C, H), dtype=np.float32) * SCALE
    Wv = rng.standard_normal((C, H), dtype=np.float32) * SCALE
    bk = rng.standard_normal(H).astype(np.float32) * 0.02
    bq = rng.standard_normal(H).astype(np.float32) * 0.02
    bv = rng.standard_normal(H).astype(np.float32) * 0.02
    out = kernel(x=x, Wk=Wk, bk=bk, Wq=Wq, bq=bq, Wv=Wv, bv=bv)
    print(out.shape, out.dtype)
